# revision 32
# baseline (speedup 1.0000x reference)
"""MultiHeadAttention Trainium2 kernel (8-core SPMD, paired AllGather).

Problem: B=4, T=2048, E=1024, H=16, D=64 multi-head self-attention
(torch-style Linear projections, softmax over keys, output projection).

Sharding: core c handles batch b=c//2 and head-half hh=c%2 (8 of the 16
heads) over ALL 2048 tokens.  Q/K/V projections and attention are
computed only for the core's own heads (no duplicated work); the final
output projection needs all heads' attention outputs, so the two cores
of each batch pair exchange their halves with a tiny 2-rank AllGather
per 512-token query block (4 x 512KB, overlapped with the next block's
attention).  Each core then produces its own 512 output columns (Wo is
column-split host-side); outputs are concatenated host-side.

Device pipeline per core (storage bf16, all accumulation fp32), emitted
as an explicitly software-pipelined "weave" over 32 units
(8 local heads x 4 query blocks) -- engines execute their instruction
streams in order, so overlap must exist at emission time:
  - per unit: 8 kc-pair score groups (K=64 matmuls, scores^T layout),
    each followed by one exp ACT [128,1024] (scale=1/8 fused, no max
    subtraction -- logits are bounded), interleaved with PV matmuls of
    a LAGGED unit (lag-2 for units 0-5 so the v projection can drain as
    fillers without starving ACT, a catch-up double-PV weave at unit 8,
    lag-1 after -- which keeps the tail to a single PV-only weave and
    emits every AllGather a unit earlier) and one filler chunk
    (v/q/k projection or output projection work) drawn from a queue,
  - PV lhsT = [v_h | 1]: psum row 64 accumulates the softmax
    denominator for free.  pv_finish first copies the UNNORMALIZED body
    out of psum (the po bank frees after ~1.3us of DVE; holding it
    through the normalize chain stalled the next PV on psum recycling
    and, in-order behind it, the next scores -> ACT gaps), then a K=1
    float32r ones-matmul broadcasts the denominator across partitions
    (hardware-proven; gpsimd partition_broadcast miscomputes on real HW
    although CoreSim accepts it) and DVE reciprocal+multiply normalize,
  - odd local heads' outputs are partition-shifted 0:64 -> 64:128 with a
    small SBUF->SBUF DMA so outT keeps a feature-major layout,
  - per query block: outT slab -> DRAM -> AllGather(pair) -> SBUF in
    rank-major head order (wot rows are in that order host-side).  The
    AllGather is split in TWO per block (head-pairs 0-1 / 2-3, each
    emitted as soon as its pv_finish lands) so each collective is ~28us
    instead of ~41us; the output projection for block qb drains one
    token-chunk per unit starting 3 units into block qb+1 (collectives
    have landed by then), so PE neither head-of-line blocks on a
    collective nor bunches outproj work into ACT-starving bursts.  The
    final block's outproj is split by AG-half dependency so its first
    fc group overlaps the last collective's flight.
"""

import os
import sys
from contextlib import ExitStack

import numpy as np
import ml_dtypes

for _p in ("/opt/trn_rl_repo", "/root/.axon_site/_ro/trn_rl_repo"):
    if os.path.isdir(_p) and _p not in sys.path:
        sys.path.insert(0, _p)

import concourse.bass as bass  # noqa: E402,F401
from concourse import bacc  # noqa: E402
import concourse.tile as tile  # noqa: E402
from concourse import mybir  # noqa: E402
from concourse.bass_utils import run_bass_kernel_spmd  # noqa: E402

# ---- problem constants (hardcoded; kernel.py must be self-contained) ----
B, T, E, H, D = 4, 2048, 1024, 16, 64
P = 128
NCORES = 8
HL = H // 2          # 8 local heads per core
FE = HL * D          # 512 local features
EC = E // P          # 8 e-chunks (contraction chunks for projections)
FCH = FE // P        # 4 local feature chunks (head pairs)
KC = T // P          # 16 key-token chunks
QB = T // 512        # 4 query blocks
TC = T // P          # 16 output token chunks

BF = mybir.dt.bfloat16
F32 = mybir.dt.float32
F32R = mybir.dt.float32r
AF = mybir.ActivationFunctionType
ALU = mybir.AluOpType

SECTIONS = []        # (name, first_instruction_index) markers for profiling
USE_TILE_POS = os.environ.get("KERNEL_NO_TILEPOS", "0") != "1"
REPEAT = int(os.environ.get("KERNEL_REPEAT", "1"))
RG_PAIRS = [[0, 1], [2, 3], [4, 5], [6, 7]]


def build_program(repeat=None):
    repeat = REPEAT if repeat is None else repeat
    nc = bacc.Bacc("TRN2", target_bir_lowering=False, debug=False,
                   num_devices=NCORES)

    def mark(name):
        SECTIONS.append((name, len(nc.inst_map)))

    xt_d = nc.dram_tensor("xt", [EC, P, T], BF, kind="ExternalInput").ap()
    wqt_d = nc.dram_tensor("wqt", [EC, P, FE], BF, kind="ExternalInput").ap()
    wkt_d = nc.dram_tensor("wkt", [EC, P, FE], BF, kind="ExternalInput").ap()
    wvt_d = nc.dram_tensor("wvt", [EC, P, FE], BF, kind="ExternalInput").ap()
    wot_d = nc.dram_tensor("wot", [EC, P, FE], BF, kind="ExternalInput").ap()
    bq_d = nc.dram_tensor("bq", [FCH, P], F32, kind="ExternalInput").ap()
    bk_d = nc.dram_tensor("bk", [FCH, P], F32, kind="ExternalInput").ap()
    bvb_d = nc.dram_tensor("bvb", [P, FE], F32, kind="ExternalInput").ap()
    bob_d = nc.dram_tensor("bob", [P, FE], F32, kind="ExternalInput").ap()
    ones_d = nc.dram_tensor("ones", [P, 64], F32R, kind="ExternalInput").ap()
    out_d = nc.dram_tensor("out", [TC, P, FE], F32,
                           kind="ExternalOutput").ap()

    with tile.TileContext(nc) as tc, ExitStack() as ctx:
        persist = ctx.enter_context(tc.tile_pool(name="persist", bufs=1))
        wq_pool = ctx.enter_context(tc.tile_pool(name="wq", bufs=2))
        wv_pool = ctx.enter_context(tc.tile_pool(name="wv", bufs=1))
        small = ctx.enter_context(tc.tile_pool(name="small", bufs=2))
        otmp_pool = ctx.enter_context(tc.tile_pool(name="otmp", bufs=2))
        fin_pool = ctx.enter_context(tc.tile_pool(name="finp", bufs=2))
        otf_pool = ctx.enter_context(tc.tile_pool(name="otf", bufs=2))
        dram = ctx.enter_context(tc.tile_pool(name="dram", bufs=4,
                                              space="DRAM"))
        psc = ctx.enter_context(tc.tile_pool(name="psc", bufs=3, space="PSUM"))
        ppv = ctx.enter_context(tc.tile_pool(name="ppv", bufs=2, space="PSUM"))

        def sc_slot():
            return psc.tile([P, 2, 512], F32, tag="sc", name="sc")

        # ---------------- persistent SBUF tensors ----------------
        xt_sb = persist.tile([P, EC, T], BF, tag="xt")          # 32K
        bq_sb = persist.tile([P, FCH], F32, tag="bq")
        bk_sb = persist.tile([P, FCH], F32, tag="bk")
        bvb_sb = persist.tile([P, FE], F32, tag="bvb")          # 2K
        bob_sb = persist.tile([P, FE], F32, tag="bob")          # 2K
        wot_sb = persist.tile([P, EC, FE], BF, tag="wot")       # 8K
        qt_sb = persist.tile([P, FCH, T], BF, tag="qt")         # 16K
        kt_sb = persist.tile([P, FCH, T], BF, tag="kt")         # 16K
        vaug = persist.tile([P, KC, HL * 65], BF, tag="vaug")   # 16.3K
        outT = persist.tile([P, FCH, T], BF, tag="outT")        # 16K
        ones_sb = persist.tile([P, 64], F32R, tag="ones")
        NEH = 3   # eh ring depth (PV lags scores by NEH-1 units)
        eh2 = [persist.tile([P, KC, 512], BF, tag=f"eh{i}", name=f"eh{i}")
               for i in range(NEH)]

        mark('in_dma')
        vaug_h = vaug.rearrange("p k (h c) -> p k h c", c=65)
        for h in range(HL):
            nc.gpsimd.memset(vaug_h[:, :, h, 64:65], 1.0)

        def emit_input_dmas():
            """Emitted AFTER qk(0)'s weight DMAs so those 0.5MB land
            first; xt then streams per-ec so the fc0 projection matmuls
            pipeline behind the (serialized) DMA engine instead of
            waiting for one monolithic 4MB transfer."""
            for ec in range(EC):
                nc.sync.dma_start(xt_sb[:, ec, :], xt_d[ec])
            nc.sync.dma_start(bq_sb[:], bq_d.rearrange("f p -> p f"))
            nc.sync.dma_start(bk_sb[:], bk_d.rearrange("f p -> p f"))
            nc.sync.dma_start(bvb_sb[:], bvb_d)
            nc.sync.dma_start(ones_sb[:], ones_d)

        # units: query-block major so each block's AllGather + output
        # projection can start as early as possible
        UNITS = [(hl, qb) for qb in range(QB) for hl in range(HL)]
        LAG = NEH - 1   # PV of unit u runs during weave of unit u+LAG

        filler = []          # queue of (key, fn); key marks a milestone
        emitted = set()      # milestone keys already drained

        def drain_filler(n=1):
            for _ in range(n):
                if not filler:
                    return
                key, fn = filler.pop(0)
                fn()
                if key is not None:
                    emitted.add(key)

        def ensure(key):
            """Drain fillers until milestone `key` has been emitted.
            Emission-order guard: a consumer instruction must be emitted
            after its producer is emitted, or the in-order engine streams
            deadlock at runtime."""
            while key not in emitted:
                assert filler, f"milestone {key} never queued"
                drain_filler(1)

        def queue_qk(fc):
            """qT/kT projection for local feature chunk fc as fillers."""
            fs = slice(fc * P, (fc + 1) * P)
            box = {}

            def dma_w():
                box["wqf"] = wq_pool.tile([P, EC, P], BF, tag="wqf",
                                          name="wqf")
                box["wkf"] = wq_pool.tile([P, EC, P], BF, tag="wkf",
                                          name="wkf")
                nc.sync.dma_start(
                    box["wqf"][:], wqt_d[:, :, fs].rearrange("e p f -> p e f"))
                nc.sync.dma_start(
                    box["wkf"][:], wkt_d[:, :, fs].rearrange("e p f -> p e f"))
            filler.append((None, dma_w))

            def group(kind, tp, last):
                w_key = "wqf" if kind == "q" else "wkf"
                dest = qt_sb if kind == "q" else kt_sb
                bias = bq_sb if kind == "q" else bk_sb
                ps_box = {}

                def mms(lo, hi):
                    def _f():
                        if "ps" not in ps_box:
                            ps_box["ps"] = sc_slot()
                        ps = ps_box["ps"]
                        for i in range(2):
                            tb = 2 * tp + i
                            for ec in range(lo, hi):
                                nc.tensor.matmul(
                                    ps[:, i, :], lhsT=box[w_key][:, ec, :],
                                    rhs=xt_sb[:, ec, tb * 512:(tb + 1) * 512],
                                    start=(ec == 0), stop=(ec == EC - 1),
                                )
                    return _f
                filler.append((None, mms(0, 4)))
                filler.append((None, mms(4, 8)))

                def evac():
                    nc.vector.tensor_scalar_add(
                        dest[:, fc, tp * 1024:(tp + 1) * 1024],
                        ps_box["ps"].rearrange("p a b -> p (a b)"),
                        bias[:, fc: fc + 1],
                    )
                filler.append((("qk", fc) if last else None, evac))
            for tp in range(T // 1024):
                group("q", tp, False)
            for tp in range(T // 1024):
                group("k", tp, tp == T // 1024 - 1)

        def queue_v():
            """v projection (token-major, +bv) as fillers; milestone
            ("v", kp) marks vaug[2kp:2kp+2] complete."""
            box = {}
            bvb_v = bvb_sb.rearrange("p (h d) -> p h d", d=D)

            def dma_w():
                box["wvh"] = wv_pool.tile([P, EC, FE], BF, tag="wvh",
                                          name="wvh")
                nc.sync.dma_start(box["wvh"][:],
                                  wvt_d.rearrange("e p f -> p e f"))
            filler.append((None, dma_w))

            for kp in range(KC // 2):
                ps_box = {}

                def mms(lo, hi, kp=kp, ps_box=ps_box):
                    def _f():
                        if "ps" not in ps_box:
                            ps_box["ps"] = sc_slot()
                        ps = ps_box["ps"]
                        for i in range(2):
                            kc = 2 * kp + i
                            for ec in range(lo, hi):
                                nc.tensor.matmul(
                                    ps[:, i, :],
                                    lhsT=xt_sb[:, ec, kc * P:(kc + 1) * P],
                                    rhs=box["wvh"][:, ec, :],
                                    start=(ec == 0), stop=(ec == EC - 1),
                                )
                    return _f
                filler.append((None, mms(0, 4)))
                filler.append((None, mms(4, 8)))

                def evac(kp=kp, ps_box=ps_box):
                    nc.vector.tensor_tensor(
                        vaug_h[:, 2 * kp: 2 * kp + 2, :, 0:64],
                        ps_box["ps"].rearrange("p a (h d) -> p a h d", d=D),
                        bvb_v[:, None, :, :].to_broadcast((P, 2, HL, D)),
                        ALU.add,
                    )
                filler.append((("v", kp), evac))

        # per-query-block AllGather machinery (two halves per block:
        # half 0 = head-pairs 0-1, half 1 = head-pairs 2-3, each emitted
        # as soon as the corresponding pv_finish has landed)
        otf_tiles = {}

        def emit_ag(qb, half):
            """outT[:, 2*half:2*half+2, qb] -> DRAM -> paired AllGather
            -> otf[:, r*FCH+2*half : +2, :] rank-major head order."""
            qs = slice(qb * 512, (qb + 1) * 512)
            hs = slice(2 * half, 2 * half + 2)
            ib = dram.tile([P, 2, 512], BF, tag="ib", name="ib")
            ob = dram.tile([2, P, 2, 512], BF, tag="ob", name="ob")
            nc.sync.dma_start(ib[:], outT[:, hs, qs])
            nc.gpsimd.collective_compute(
                "AllGather", ALU.bypass,
                replica_groups=RG_PAIRS,
                ins=[ib.opt()], outs=[ob.opt()],
            )
            if half == 0:
                otf_tiles[qb] = otf_pool.tile([P, 2 * FCH, 512], BF,
                                              tag="otf", name="otf")
            otf = otf_tiles[qb]
            for r in range(2):
                nc.sync.dma_start(
                    otf[:, r * FCH + 2 * half: r * FCH + 2 * half + 2, :],
                    ob[r])

        # outproj fc groups by AG-half dependency (rank-major otf layout:
        # fc r*FCH+hp; half 0 delivers hp 0,1 for both ranks)
        OP_A = [0, 1, 4, 5]
        OP_B = [2, 3, 6, 7]

        def op_chunks(qb, tcl):
            """(A_mms, B_mms, evac) filler fns for token chunk tcl of
            block qb.  A depends only on AG(qb,0); B on AG(qb,1)."""
            otf = otf_tiles[qb]
            tc_ = qb * 4 + tcl
            ps_box = {}

            def mms(fcs, start, stop):
                def _f():
                    if "ps" not in ps_box:
                        ps_box["ps"] = sc_slot()
                    ps = ps_box["ps"]
                    for j, fc in enumerate(fcs):
                        nc.tensor.matmul(
                            ps[:, 0, :],
                            lhsT=otf[:, fc, tcl * P:(tcl + 1) * P],
                            rhs=wot_sb[:, fc, :],
                            start=(start and j == 0),
                            stop=(stop and j == len(fcs) - 1),
                        )
                return _f

            def evac():
                fin = fin_pool.tile([P, FE], F32, tag="fin", name="fin")
                nc.vector.tensor_tensor(
                    fin[:], ps_box["ps"][:, 0, :], bob_sb[:], ALU.add)
                nc.sync.dma_start(out_d[tc_], fin[:])
            return (mms(OP_A, True, False), mms(OP_B, False, True), evac)

        def queue_outproj_tc(qb, tcl):
            """final[t, own j cols] for one token chunk of block qb --
            queued one tc per unit so outproj PE work spreads across the
            block instead of bunching up and starving ACT."""
            a, b, ev = op_chunks(qb, tcl)
            filler.append((None, a))
            filler.append((None, b))
            filler.append((None, ev))

        def queue_outproj_tail(qb):
            """Tail version: A chunks for tc 0-2 first (they only wait on
            AG(qb,0), long landed, and overlap AG(qb,1)'s flight); the
            rest ordered so no PE instruction ever waits, via a psum-slot
            reuse or otf, on work emitted after it."""
            parts = [op_chunks(qb, tcl) for tcl in range(4)]
            pre = [("A", parts[0][0]), ("A", parts[1][0]), ("A", parts[2][0])]
            post = [parts[0][1], parts[0][2],        # B0, ev0
                    parts[3][0],                     # A3 (psum slot of tc0)
                    parts[1][1], parts[1][2],        # B1, ev1
                    parts[2][1], parts[2][2],        # B2, ev2
                    parts[3][1], parts[3][2]]        # B3, ev3
            for _, fn in pre:
                filler.append((None, fn))
            return post

        pv_state = {}

        def pv_mms(ui, kc):
            hl, qb = UNITS[ui]
            nc.tensor.matmul(
                pv_state[ui]["po"][0:65, :], lhsT=vaug_h[:, kc, hl, :],
                rhs=eh2[ui % NEH][:, kc, :],
                start=(kc == 0), stop=(kc == KC - 1),
            )

        def pv_finish(ui):
            hl, qb = UNITS[ui]
            hp, par = hl // 2, hl % 2
            qs = slice(qb * 512, (qb + 1) * 512)
            po = pv_state.pop(ui)["po"]
            srb = small.tile([P, 512], F32R, tag="srb", name="srb")
            nc.vector.tensor_copy(srb[64:65, :], po[64:65, :])
            # evacuate the UNNORMALIZED body immediately -- po's psum
            # bank frees after ~1.3us of DVE instead of after the full
            # broadcast->reciprocal->multiply chain, whose ~5us latency
            # stalled the next-next unit's PV matmuls on ppv recycling
            # (and behind them, in-order, the next scores -> ACT gaps)
            un = otmp_pool.tile([P, 512], BF, tag="un", name="un")
            nc.vector.tensor_copy(un[0:64, :], po[0:64, :])
            # denominator row -> partitions 0:64 via K=1 float32r ones
            # matmul (hardware-proven; gpsimd partition_broadcast gives
            # wrong results on HW although CoreSim accepts it)
            psR = sc_slot()
            nc.tensor.matmul(psR[0:64, 0, :], lhsT=ones_sb[64:65, :],
                             rhs=srb[64:65, :], start=True, stop=True)
            with nc.allow_low_precision(
                    reason="float32r is bit-identical fp32 storage"):
                nc.vector.reciprocal(srb[0:64, :], psR[0:64, 0, :])
            if par == 0:
                nc.vector.tensor_tensor(outT[0:64, hp, qs], un[0:64, :],
                                        srb[0:64, :], ALU.mult)
            else:
                ot = otmp_pool.tile([P, 512], BF, tag="ot", name="ot")
                nc.vector.tensor_tensor(ot[0:64, :], un[0:64, :],
                                        srb[0:64, :], ALU.mult)
                nc.sync.dma_start(outT[64:128, hp, qs], ot[0:64, :])

        def pv_for(ui):
            """PV units to process during weave ui.  Lag-2 while the v
            projection drains as fillers (units 0-5), then a catch-up
            weave at unit 8 (PV 6+7 together still fits under the ACT
            pace: 6 matmuls/group < 1 exp) and lag-1 from there on --
            lag-1 keeps the tail to a single PV-only weave and emits
            every AllGather one unit earlier."""
            if ui < 2:
                return []
            if ui <= 7:
                return [ui - 2]
            if ui == 8:
                return [6, 7]
            return [ui - 1]

        def weave_unit(ui, pvs=()):
            """Emit unit ui's scores+exp interleaved with PV matmuls of
            units `pvs` and filler chunks (ui=None: PV/drain only)."""
            pvs = [u for u in pvs if u in pv_state]
            if ui is not None:
                hl, qb = UNITS[ui]
                hp, par = hl // 2, hl % 2
                ensure(("qk", hp))
                qs = slice(qb * 512, (qb + 1) * 512)
                rows = slice(0, 64) if par == 0 else slice(64, 128)
                tp = (dict(tile_position=(0, 0)) if par == 0 else
                      dict(tile_position=(64, 0))) if USE_TILE_POS else {}
                eh = eh2[ui % NEH]
            for g in range(KC // 2):
                if ui is not None:
                    ps2 = sc_slot()
                    for i in range(2):
                        kc = 2 * g + i
                        kslc = slice(kc * P, (kc + 1) * P)
                        nc.tensor.matmul(
                            ps2[:, i, :], lhsT=kt_sb[rows, hp, kslc],
                            rhs=qt_sb[rows, hp, qs],
                            start=True, stop=True, **tp,
                        )
                    nc.scalar.activation(eh[:, 2 * g: 2 * g + 2, :], ps2[:],
                                         AF.Exp, scale=0.125)
                if pvs:
                    ensure(("v", g))
                    for u in pvs:
                        pv_mms(u, 2 * g)
                        pv_mms(u, 2 * g + 1)
                drain_filler(1)
            for u in pvs:
                pv_finish(u)
                hlp, qbp = UNITS[u]
                if hlp == HL // 2 - 1:
                    emit_ag(qbp, 0)
                elif hlp == HL - 1:
                    emit_ag(qbp, 1)
            if ui is not None:
                pv_state[ui] = {
                    "po": ppv.tile([P, 512], F32, tag="po", name="po")}

        def emit_body():
            # bootstrap: first feature chunk's projections + first unit's
            # scores; v projection + remaining q/k feature chunks drain as
            # weave fillers (PV lags LAG units to give them room)
            emitted.clear()
            queue_qk(0)
            drain_filler(1)      # wqf/wkf fc0 weight DMAs first
            emit_input_dmas()
            drain_filler(100)
            queue_qk(1)
            queue_v()

            mark('attention')
            nc.sync.dma_start(wot_sb[:], wot_d.rearrange("e p f -> p e f"))
            nc.sync.dma_start(bob_sb[:], bob_d)

            for ui in range(len(UNITS)):
                if ui == 2:
                    queue_qk(2)
                elif ui == 4:
                    queue_qk(3)
                if ui >= 11 and (ui - 11) % 8 < 4 and (ui - 11) // 8 < QB - 1:
                    # both AG(qb) halves have landed (emitted at weaves
                    # 8qb+4 / 8qb+8, ~28us flight) -> outproj mms won't
                    # camp on a psum slot waiting for a collective
                    queue_outproj_tc((ui - 11) // 8, (ui - 11) % 8)
                weave_unit(ui, pv_for(ui))
            # tail: one PV-only weave finishes unit 31 and launches the
            # final AllGather half; only THEN drain outproj chunks (the
            # A group overlaps that collective's flight, B follows it)
            weave_unit(None, pvs=[len(UNITS) - 1])
            post = queue_outproj_tail(QB - 1)   # A chunks for tc 0-2
            drain_filler(100)
            for fn in post:
                filler.append((None, fn))
            drain_filler(100)

        for _rep in range(repeat):
            emit_body()

        mark('tail')
    nc.compile()
    return nc


_NC = None


def _get_nc():
    global _NC
    if _NC is None:
        _NC = build_program()
    return _NC


def _prep_core_inputs(x, Wq, bq, Wk, bk, Wv, bv, Wo, bo):
    """Build the 8 per-core input dicts (host-side sharding)."""
    bf = ml_dtypes.bfloat16
    x = np.asarray(x, dtype=np.float32)
    Wq, Wk, Wv, Wo = (np.asarray(a, np.float32) for a in (Wq, Wk, Wv, Wo))
    bq, bk, bv, bo = (np.asarray(a, np.float32) for a in (bq, bk, bv, bo))
    ones_a = np.ones((P, 64), np.float32)

    halves = []
    for hh in range(2):
        fs = slice(hh * FE, (hh + 1) * FE)
        wqt = np.ascontiguousarray(Wq.T[:, fs]).astype(bf).reshape(EC, P, FE)
        wkt = np.ascontiguousarray(Wk.T[:, fs]).astype(bf).reshape(EC, P, FE)
        wvt = np.ascontiguousarray(Wv.T[:, fs]).astype(bf).reshape(EC, P, FE)
        # wot: all E feature rows (natural order = rank-major), own j cols
        wot = np.ascontiguousarray(Wo.T[:, fs]).astype(bf).reshape(EC, P, FE)
        bq_a = np.ascontiguousarray(bq[fs]).reshape(FCH, P)
        bk_a = np.ascontiguousarray(bk[fs]).reshape(FCH, P)
        bvb = np.ascontiguousarray(
            np.broadcast_to(bv[fs][None, :], (P, FE)))
        bob = np.ascontiguousarray(
            np.broadcast_to(bo[fs][None, :], (P, FE)))
        halves.append(dict(wqt=wqt, wkt=wkt, wvt=wvt, wot=wot, bq=bq_a,
                           bk=bk_a, bvb=bvb, bob=bob))

    in_maps = []
    for c in range(NCORES):
        b, hh = c // 2, c % 2
        hv = halves[hh]
        xt = np.ascontiguousarray(x[b].T).astype(bf).reshape(EC, P, T)
        in_maps.append({
            "xt": xt, "wqt": hv["wqt"], "wkt": hv["wkt"], "wvt": hv["wvt"],
            "wot": hv["wot"], "bq": hv["bq"], "bk": hv["bk"],
            "bvb": hv["bvb"], "bob": hv["bob"], "ones": ones_a,
        })
    return in_maps


def kernel(x, Wq, bq, Wk, bk, Wv, bv, Wo, bo):
    nc = _get_nc()
    in_maps = _prep_core_inputs(x, Wq, bq, Wk, bk, Wv, bv, Wo, bo)
    res = run_bass_kernel_spmd(nc, in_maps, list(range(NCORES)))
    out = np.empty((B, T, E), np.float32)
    for c in range(NCORES):
        b, hh = c // 2, c % 2
        out[b, :, hh * FE:(hh + 1) * FE] = res.results[c]["out"].reshape(T, FE)
    return out

